# revision 1
# baseline (speedup 1.0000x reference)
"""DGAT attention head on 8 trn2 NeuronCores.

Sharding: row-wise over query nodes (core c owns rows [c*R, (c+1)*R)).
Each core receives its adj slice pre-transposed and mask-encoded
(host-side layout choice): adjt_enc = (adj^T - 1) * BIG in bf16
(exact: adj is binary), so masking becomes an additive logit bias.

Math (exact for binary adj):
  h   = x @ w;  hz1 = x @ (w @ a[:D]);  hz2 = x @ (w @ a[D:])
  z   = C*(hz1[i] + hz2[j]) + D0;  L1 = leaky(A+B)   (leaky slope 0.2)
  row-softmax of masked logits  ==  normalize(exp(L1*leaky(z) - G + BIG*(adj-1)))
  out = elu((p @ h) / (p @ 1))
G is a host-derived bound on max logit (from hz1/hz2 extremes).

Device pipeline per 256-j megatile (j on partitions, i free):
  DMA 512KB bf16 adjt_enc
  -> custom DVE op: u = leakyscaled(Src0 + bias_col) + adjt_enc - G   (1 pass)
  -> ACT Exp: q = exp(u), bf16 out                                    (1 pass)
  -> PE: psum[65, R] += [h|1]^T-group @ q-half (bf16, N=1024)
Tail: psum -> sbuf, PE transposes [65,128]->[128,65], reciprocal of the
sum column, ELU = relu(v) + (min(exp(v),1) - 1).

The leaky-scale trick: for L1>=0, u_leak = select(w>=0, w, 0.2w) with
w = L1*z (positive homogeneity); for L1<0, w = 0.2*L1*z and the false
branch multiplies by 1/0.2.
"""

import numpy as np
import ml_dtypes

import concourse.bass as bass
import concourse.bacc as bacc
import concourse.mybir as mybir
import concourse.dve_ops as dve_ops
from concourse.dve_spec import Spec, Src0, Src1, C0, C1, C2, Zero, One, select, maxx
from concourse.tile import TileContext
from concourse.bass_utils import run_bass_kernel_spmd

F32 = mybir.dt.float32
F16 = mybir.dt.float16
F32R = mybir.dt.float32r
F8 = mybir.dt.float8e5
AF = mybir.ActivationFunctionType
OP = mybir.AluOpType

NCORES = 8
SLOPE = 0.2    # leakyrelu negative slope (fixed in the reference)
BIG = 16384.0  # additive mask magnitude (exact in fp16; exp(-BIG) == 0)

TRACE = False
LAST_RESULTS = None
LAST_NC = None


def _leaky(z):
    return z if z >= 0.0 else SLOPE * z


def _register_leaky_mask_op():
    name = "LEAKY_MASK_BIAS_ANT"
    for op in dve_ops.OPS:
        if op.name == name:
            return op
    w = Src0 + C0
    spec = Spec(
        body=select(w >= Zero, w, w * C1) + Src1 + C2,
        reference=lambda in0, in1, s0, s1, imm2: (
            np.where(in0 + s0 >= 0, in0 + s0, (in0 + s0) * s1) + in1 + imm2
        ).astype(np.float32),
    )
    return _finish_register(name, spec)


def _finish_register(name, spec):
    from concourse.dve_spec import lower
    from concourse.dve_ops import has_src1
    from concourse.dve_uop import DveOpSpec

    op = dve_ops.DveOp(name, spec, subdim=False, uops_sha={})
    dve_ops.OPS.append(op)
    dve_ops.CUSTOM_DVE_SPECS[name] = spec
    dve_ops._SUB_OPCODE_FOR_NAME[name] = (
        dve_ops._CUSTOM_DVE_ROW_BASE + len(dve_ops.OPS) - 1
    )
    assert dve_ops._SUB_OPCODE_FOR_NAME[name] < 0x20
    for ver in ("v3",):
        pinned = DveOpSpec(
            name=name,
            opcode=dve_ops.get_dve_sub_opcode(name),
            uops=lower(spec, ver=ver),
            rd1_en=has_src1(spec),
        ).sha(ver)
        op.uops_sha[ver] = pinned
        dve_ops._COMPILE_CACHE.pop((name, ver), None)
        op.compile(ver)
    return op


def _register_elu_max_op():
    name = "ELU_MAX_ANT"
    for op in dve_ops.OPS:
        if op.name == name:
            return op
    spec = Spec(
        body=maxx(Src0 * C0, Src1 - One),
        reference=lambda in0, in1, s0, s1, imm2: np.maximum(
            in0 * s0, in1 - 1.0
        ).astype(np.float32),
    )
    return _finish_register(name, spec)


def _build(n, din, dout, rows, kpre, s1_slope, G):
    """Build the SPMD Bass program (identical on all cores).

    kpre: scale applied to hz1/hz2 logit halves (= k*C with k = L1 or
    SLOPE*L1); the per-partition bias col is kpre*hz2 + kD (kD folded on
    device); s1_slope: false-branch slope of the select (0.2 or 5.0).
    """
    assert n % 256 == 0 and rows % 128 == 0 and din % 128 == 0
    ng = n // 128
    mt = 4 if n % 512 == 0 else 2
    nm = n // (128 * mt)
    kc = din // 128
    grp = 4
    assert ng % grp == 0
    de = dout + 1
    lmb = _register_leaky_mask_op()
    emx = _register_elu_max_op()

    nc = bacc.Bacc("TRN2", target_bir_lowering=False)
    adjt_d = nc.dram_tensor("adjt", [n, rows], F8, kind="ExternalInput")
    xt_d = nc.dram_tensor("xt", [din, n], F16, kind="ExternalInput")
    xto_d = nc.dram_tensor("xt_own", [din, rows], F16, kind="ExternalInput")
    w_d = nc.dram_tensor("w", [din, dout], F32, kind="ExternalInput")
    a_d = nc.dram_tensor("a", [2 * dout, 1], F32, kind="ExternalInput")
    kd_d = nc.dram_tensor("kd", [1, 1], F32, kind="ExternalInput")
    y_d = nc.dram_tensor("y", [rows, dout], F32, kind="ExternalOutput")

    with TileContext(nc) as tc:
        with (
            tc.tile_pool(name="consts", bufs=1) as consts,
            tc.tile_pool(name="adjp", bufs=8) as adjp,
            tc.tile_pool(name="up", bufs=2) as up,
            tc.tile_pool(name="qp", bufs=2) as qp,
            tc.tile_pool(name="et2p", bufs=1) as et2p,
            tc.tile_pool(name="tailp", bufs=2) as tailp,
        ):
            from concourse.masks import make_identity

            identity0 = consts.tile([128, 128], F32)
            make_identity(nc, identity0)
            identity = consts.tile([128, 128], F32)
            nc.vector.tensor_copy(identity, identity0)

            zcol = consts.tile([128, 1], F32)
            nc.vector.memset(zcol, 0.0)
            negGcol = consts.tile([128, 1], F32)
            nc.vector.memset(negGcol, -G)
            ones128 = consts.tile([128, 128], F16)
            nc.vector.memset(ones128, 1.0)
            # kD broadcast column (k*D0 replicated to all partitions)
            kdcol = consts.tile([128, 1], F32)
            kd_ap = kd_d[:, :]
            nc.sync.dma_start(
                out=kdcol,
                in_=bass.AP(tensor=kd_ap.tensor, offset=0, ap=[[0, 128], [1, 1]]),
            )

            # a1/a2 broadcast across partitions (partition-step-0 DMA)
            a_ap = a_d[:, :]
            a1bc = consts.tile([128, dout], F32)
            nc.sync.dma_start(
                out=a1bc,
                in_=bass.AP(tensor=a_ap.tensor, offset=0, ap=[[0, 128], [1, dout]]),
            )
            a2bc = consts.tile([128, dout], F32)
            nc.sync.dma_start(
                out=a2bc,
                in_=bass.AP(
                    tensor=a_ap.tensor, offset=dout, ap=[[0, 128], [1, dout]]
                ),
            )

            # wx_k = [w_k | w_k@a1 | w_k@a2] in bf16, single DVE writer
            wx = []
            wxraw = []
            for k in range(kc):
                wxr = consts.tile([128, dout + 2], F32, name=f"wxr{k}")
                nc.sync.dma_start(
                    out=wxr[:, 0:dout], in_=w_d[k * 128 : (k + 1) * 128, :]
                )
                t1 = consts.tile([128, dout], F32, name=f"wa_t{k}")
                nc.vector.tensor_mul(t1, wxr[:, 0:dout], a1bc)
                nc.vector.reduce_sum(
                    wxr[:, dout : dout + 1], t1, axis=mybir.AxisListType.X
                )
                t2 = consts.tile([128, dout], F32, name=f"wb_t{k}")
                nc.vector.tensor_mul(t2, wxr[:, 0:dout], a2bc)
                nc.vector.reduce_sum(
                    wxr[:, dout + 1 : dout + 2], t2, axis=mybir.AxisListType.X
                )
                wxk = consts.tile([128, dout + 2], F16, name=f"wx{k}")
                nc.vector.tensor_copy(wxk, wxr)
                wx.append(wxk)
                wxraw.append(wxr)

            h_ext = consts.tile([128, ng, de], F32R)
            # memset can't write f32r; copy from a ones tile instead
            nc.vector.tensor_copy(
                h_ext[:, :, dout : dout + 1], ones128[:, 0:ng]
            )
            hz1bc = consts.tile([128, rows], F32)
            hz2cols = consts.tile([128, ng], F32)
            bias_cols = consts.tile([128, ng], F32)
            hpT = consts.tile([de, rows], F32)

            with (
                tc.tile_pool(name="xtp", bufs=1) as xtp,
                tc.tile_pool(name="pshz", bufs=1, space="PSUM") as pshz,
                tc.tile_pool(name="pspre", bufs=2, space="PSUM") as pspre,
            ):
                # own-x columns + hz1 broadcast first: this unblocks the
                # main-loop custom ops as early as possible
                xtos = []
                for k in range(kc):
                    xtok = xtp.tile([128, rows], F16, name=f"xto{k}")
                    nc.sync.dma_start(
                        out=xtok, in_=xto_d[k * 128 : (k + 1) * 128, :]
                    )
                    xtos.append(xtok)
                hz_ps = pshz.tile([128, rows], F32)
                for k in range(kc):
                    wa1bc = consts.tile([128, 128], F16, name=f"wa1bc{k}")
                    nc.vector.tensor_scalar_mul(
                        wa1bc, ones128, wxraw[k][:, dout : dout + 1]
                    )
                    nwmax = 512
                    for n0 in range(0, rows, nwmax):
                        nw = min(nwmax, rows - n0)
                        nc.tensor.matmul(
                            hz_ps[:, n0 : n0 + nw],
                            wa1bc,
                            xtos[k][:, n0 : n0 + nw],
                            start=(k == 0),
                            stop=(k == kc - 1),
                        )
                nc.vector.tensor_scalar_mul(hz1bc, hz_ps, kpre)

                xchunk = 2048 if n >= 2048 else n
                xts = [
                    xtp.tile([128, n], F16, name=f"xt{k}") for k in range(kc)
                ]
                for c0 in range(0, n, xchunk):
                    for k in range(kc):
                        nc.sync.dma_start(
                            out=xts[k][:, c0 : c0 + xchunk],
                            in_=xt_d[k * 128 : (k + 1) * 128, c0 : c0 + xchunk],
                        )

                # h_ext (f32r), hz2 and bias columns per j-group, in the
                # order the main loop consumes them
                for g0 in range(0, ng, grp):
                    ps = pspre.tile([128, grp, dout + 2], F32, name="ps_h")
                    for gi in range(grp):
                        g = g0 + gi
                        for k in range(kc):
                            nc.tensor.matmul(
                                ps[:, gi, :],
                                xts[k][:, g * 128 : (g + 1) * 128],
                                wx[k],
                                start=(k == 0),
                                stop=(k == kc - 1),
                            )
                    nc.scalar.copy(
                        h_ext[:, g0 : g0 + grp, 0:dout], ps[:, :, 0:dout]
                    )
                    nc.scalar.copy(
                        hz2cols[:, g0 : g0 + grp],
                        ps[:, :, dout + 1 : dout + 2],
                    )
                    nc.vector.tensor_scalar(
                        bias_cols[:, g0 : g0 + grp],
                        hz2cols[:, g0 : g0 + grp],
                        kpre,
                        kdcol[:, 0:1],
                        OP.mult,
                        OP.add,
                    )

            # main loop: stream encoded adjT megatiles (256 j x rows i)
            adjt_r = adjt_d[:, :].rearrange(
                "(m t p) i -> m p t i", t=mt, p=128
            )
            with (
                tc.tile_pool(name="psacc", bufs=1, space="PSUM") as psacc,
                tc.tile_pool(name="pstail", bufs=4, space="PSUM") as pstail,
            ):
                acc = psacc.tile([de, rows], F32)
                # last megatile runs leaky on ACT (Prelu) + mask-add on
                # GPSIMD to relieve the DVE bottleneck
                gp_ms = set()  # GP-assist measured slower in timeline sim
                for m in range(nm):
                    adjt_t = adjp.tile([128, mt * rows], F8)
                    nc.sync.dma_start(
                        out=adjt_t.rearrange("p (t i) -> p t i", t=mt),
                        in_=adjt_r[m],
                    )
                    u = up.tile([128, mt * rows], F32)
                    if m in gp_ms:
                        et2 = et2p.tile([128, mt * rows], F32)
                        for t in range(mt):
                            g = mt * m + t
                            nc.scalar.activation(
                                et2[:, t * rows : (t + 1) * rows],
                                hz1bc,
                                AF.Prelu,
                                bias=bias_cols[:, g : g + 1],
                                alpha=s1_slope,
                            )
                        for t in range(mt):
                            sl = slice(t * rows, (t + 1) * rows)
                            nc.gpsimd.tensor_add(
                                u[:, sl], et2[:, sl], adjt_t[:, sl]
                            )
                        expbias = negGcol
                    else:
                        for t in range(mt):
                            g = mt * m + t
                            nc.vector._custom_dve(
                                lmb,
                                out=u[:, t * rows : (t + 1) * rows],
                                in0=hz1bc,
                                in1=adjt_t[:, t * rows : (t + 1) * rows],
                                s0=bias_cols[:, g : g + 1],
                                s1=s1_slope,
                                imm2=-G,
                            )
                        expbias = zcol
                    q = qp.tile([128, mt * rows], F32R)
                    for t0 in range(0, mt, 2):
                        nc.scalar.activation(
                            q[:, t0 * rows : (t0 + 2) * rows],
                            u[:, t0 * rows : (t0 + 2) * rows],
                            AF.Exp,
                            bias=expbias[:, 0:1],
                        )
                    for t in range(mt):
                        g = mt * m + t
                        nwmax = 512
                        for n0 in range(0, rows, nwmax):
                            nw = min(nwmax, rows - n0)
                            nc.tensor.matmul(
                                acc[:, n0 : n0 + nw],
                                h_ext[:, g, :],
                                q[:, t * rows + n0 : t * rows + n0 + nw],
                                start=(g == 0),
                                stop=(g == ng - 1),
                            )

                # tail: normalize + elu, back to i-major.
                # hpT row de holds 1/s so each transposed chunk carries its
                # per-partition reciprocal in column de.
                nc.scalar.copy(hpT[0:dout, :], acc[0:dout, :])
                nc.vector.reciprocal(hpT[dout:de, :], acc[dout:de, :])
                for cc in range(rows // 128):
                    tp = pstail.tile([128, de], F32)
                    nc.tensor.transpose(
                        tp,
                        hpT[:, cc * 128 : (cc + 1) * 128],
                        identity[0:de, 0:de],
                    )
                    # elu(v) = max(v, exp(min(v, 0)) - 1), v = hp * (1/s)
                    vm = tailp.tile([128, dout], F32)
                    nc.vector.tensor_scalar(
                        vm, tp[:, 0:dout], tp[:, dout:de], 0.0,
                        OP.mult, OP.min,
                    )
                    e2 = tailp.tile([128, dout], F32)
                    nc.scalar.activation(e2, vm, AF.Exp, bias=zcol[:, 0:1])
                    ysb = tailp.tile([128, dout], F32)
                    nc.vector._custom_dve(
                        emx, out=ysb, in0=tp[:, 0:dout], in1=e2,
                        s0=tp[:, dout:de], s1=0.0, imm2=0.0,
                    )
                    nc.sync.dma_start(
                        out=y_d[cc * 128 : (cc + 1) * 128, :], in_=ysb
                    )
    nc.compile()
    return nc


def _run(x, adj, w, a, a_coeff, b_coeff, c_coeff, d_coeff):
    global LAST_RESULTS, LAST_NC
    n, din = x.shape
    dout = w.shape[1]
    assert adj.shape == (n, n) and a.shape == (2 * dout, 1)
    rows = n // NCORES

    A = float(np.asarray(a_coeff).reshape(-1)[0])
    B = float(np.asarray(b_coeff).reshape(-1)[0])
    C = float(np.asarray(c_coeff).reshape(-1)[0])
    D0 = float(np.asarray(d_coeff).reshape(-1)[0])
    L1 = _leaky(A + B)

    x = np.ascontiguousarray(x, dtype=np.float32)
    adj = np.asarray(adj, dtype=np.float32)
    # the mask-encoding algebra requires a binary adjacency
    assert ((adj == 0.0) | (adj == 1.0)).all(), "adj must be binary"
    w = np.ascontiguousarray(w, dtype=np.float32)
    a = np.ascontiguousarray(a, dtype=np.float32)

    # host-side stability shift G >= max logit (from h extremes only)
    h = x @ w
    hz1 = h @ a[:dout, 0]
    hz2 = h @ a[dout:, 0]
    cand = []
    for u in (hz1.min(), hz1.max()):
        for v in (hz2.min(), hz2.max()):
            cand.append(L1 * _leaky(C * (float(u) + float(v)) + D0))
    G = float(max(cand))

    # leaky-scale trick (positive homogeneity of leaky)
    if L1 >= 0.0:
        kk, s1_slope = L1, SLOPE
    else:
        kk, s1_slope = SLOPE * L1, 1.0 / SLOPE
    kpre = kk * C

    nc = _build(n, din, dout, rows, kpre, s1_slope, G)
    LAST_NC = nc

    xt_b = np.ascontiguousarray(x.T).astype(np.float16)
    kd = np.full((1, 1), kk * D0, dtype=np.float32)
    in_maps = []
    for c in range(NCORES):
        sl = slice(c * rows, (c + 1) * rows)
        adjt_enc = ((adj[sl, :].T - 1.0) * BIG).astype(ml_dtypes.float8_e5m2)
        in_maps.append(
            {
                "adjt": np.ascontiguousarray(adjt_enc),
                "xt": xt_b,
                "xt_own": np.ascontiguousarray(xt_b[:, sl]),
                "w": w,
                "a": a,
                "kd": kd,
            }
        )

    res = run_bass_kernel_spmd(
        nc, in_maps, core_ids=list(range(NCORES)), trace=TRACE
    )
    LAST_RESULTS = res
    return np.concatenate([r["y"] for r in res.results], axis=0)


def kernel(x, adj, w, a, a_coeff, b_coeff, c_coeff, d_coeff):
    return _run(x, adj, w, a, a_coeff, b_coeff, c_coeff, d_coeff)



# revision 6
# speedup vs baseline: 1.3868x; 1.3868x over previous
"""DGAT attention head on 8 trn2 NeuronCores.

Sharding: row-wise over query nodes (core c owns rows [c*R, (c+1)*R)).

Math (exact reparameterization of the reference):
  logit[i,j] = L1*leaky(z), z = C*(hz1_i+hz2_j)+D, L1 = leaky(A+B)
  exp(logit) factorizes per leaky branch; with L1>0 the branch select is a
  max, and softmax rows are scale-invariant, so dividing row i by
  exp(L1*C*hz1_i) leaves
      q[j,i] = adj[j,i] * max(P2[j], rho[i]*N2[j])
  with per-node vectors P2 = exp(a_e*hz2 + L1*D + lng),
  N2 = exp(b_e*hz2 + s*L1*D + lng), rho = exp((b_e-a_e)*hz1),
  a_e = L1*C, b_e = s*L1*C, s = 0.2 (lng: global scale for f16 range).
  No full-matrix exp/leaky remains; host precomputes h, rho, P2, N2.

Device pipeline per 128j x R megatile, split across four engine paths to
balance DVE / ACT / Pool / DMA occupancy (PE runs the shared accumulation):
  A: DVE  t = rho_bc * N2col            (f16 tensor_scalar, 4x mode)
     Pool q = max(t, P2col) * adjA      (scalar_tensor_tensor, fp8 adj)
  B: DVE  m = (rho_bc * N2col) max P2col  (dual-scalar tensor_scalar, 4x)
     DVE  q = m * adjF                  (tensor_tensor f16, 2x mode)
  D: ACT  v = Relu(n2s*rho_bc + b1)     (per-partition scale/bias cols)
     ACT  m = Identity(ms*v + P2col)
     DVE  q = m * adjF                  (tensor_tensor f16, 2x mode)
  E: DVE  q = max(max(P2col, rho_bc*N2col) + encE, 0)  (fused custom op,
     additive fp8 mask encoding {0, -BIG})
  PE: acc[65, R] += [h|1]^T_g @ q       (f16, start/stop over all 64 groups)
Emission is software-pipelined: D's mask op and the PE matmuls lag their
producers by a few steps so no in-order engine queue head-blocks on a
cross-engine dependency.
Tail: reciprocal of the sum row, PE transpose back to i-major,
  elu(v) = max(v, exp(min(v,0)) - 1), emitted engine-major.
"""

import numpy as np
import ml_dtypes

import concourse.bass as bass
import concourse.bacc as bacc
import concourse.mybir as mybir
import concourse.dve_ops as dve_ops
from concourse.dve_spec import Spec, Src0, Src1, C0, C1, C2, Zero, One, select, maxx
from concourse.tile import TileContext
from concourse.bass_utils import run_bass_kernel_spmd

F32 = mybir.dt.float32
F16 = mybir.dt.float16
F8 = mybir.dt.float8e5
AF = mybir.ActivationFunctionType
OP = mybir.AluOpType

NCORES = 8
SLOPE = 0.2  # leakyrelu negative slope (fixed in the reference)
BIG = 32768.0  # additive mask magnitude for path E (exact in fp8e5/f16)

# problem dims (hardcoded per spec)
N, DIN, DOUT = 8192, 256, 64
R = N // NCORES          # 1024 query rows per core
NG = N // 128            # 64 j-groups
DE = DOUT + 1            # h columns + ones column

CHUNK = 4                # j-groups per adjacency DMA
D_LAG = 3                # D-path mask op lags its ACT producers (in D-steps)
PE_LAG = 3               # matmuls lag q production (in steps)

TRACE = False
LAST_RESULTS = None
LAST_NC = None


def _leaky(z):
    return z if z >= 0.0 else SLOPE * z


def _finish_register(name, spec):
    from concourse.dve_spec import lower
    from concourse.dve_ops import has_src1
    from concourse.dve_uop import DveOpSpec

    op = dve_ops.DveOp(name, spec, subdim=False, uops_sha={})
    dve_ops.OPS.append(op)
    dve_ops.CUSTOM_DVE_SPECS[name] = spec
    dve_ops._SUB_OPCODE_FOR_NAME[name] = (
        dve_ops._CUSTOM_DVE_ROW_BASE + len(dve_ops.OPS) - 1
    )
    assert dve_ops._SUB_OPCODE_FOR_NAME[name] < 0x20
    for ver in ("v3",):
        pinned = DveOpSpec(
            name=name,
            opcode=dve_ops.get_dve_sub_opcode(name),
            uops=lower(spec, ver=ver),
            rd1_en=has_src1(spec),
        ).sha(ver)
        op.uops_sha[ver] = pinned
        dve_ops._COMPILE_CACHE.pop((name, ver), None)
        op.compile(ver)
    return op


def _register_elu_max_op():
    name = "ELU_MAX_ANT"
    for op in dve_ops.OPS:
        if op.name == name:
            return op
    spec = Spec(
        body=maxx(Src0 * C0, Src1 - One),
        reference=lambda in0, in1, s0, s1, imm2: np.maximum(
            in0 * s0, in1 - 1.0
        ).astype(np.float32),
    )
    return _finish_register(name, spec)


def _register_maxmax_op():
    name = "MAXMAX2_ANT"
    for op in dve_ops.OPS:
        if op.name == name:
            return op
    spec = Spec(
        body=maxx(maxx(C0, Src1 * C1) + Src0, Zero),
        reference=lambda in0, in1, s0, s1, imm2: np.maximum(
            np.maximum(s0, in1 * s1) + in0, 0.0
        ).astype(np.float32),
    )
    return _finish_register(name, spec)


def _interleave(counts):
    """Largest-remainder round-robin over path labels."""
    tot = sum(counts.values())
    acc = {k: 0.0 for k in counts}
    out = []
    for _ in range(tot):
        for k in counts:
            acc[k] += counts[k] / tot
        k = max(acc, key=lambda kk: acc[kk])
        acc[k] -= 1.0
        out.append(k)
    return out


def _build(op_sel, mid_sign, na, nb, nd, ne):
    """Build the SPMD Bass program (identical on all cores).

    op_sel: OP.max when L1 >= 0 else OP.min (branch-select direction).
    mid_sign: +1.0 / -1.0 scale on the D-path Identity op (min variant
    computes m = P2 - relu(P2 - rho*N2)). na/nb/nd/ne: path tile counts.
    """
    assert na + nb + nd + ne == NG
    emx = _register_elu_max_op()
    mmx = _register_maxmax_op()

    nc = bacc.Bacc("TRN2", target_bir_lowering=False)
    adjA_d = nc.dram_tensor("adjA", [max(na, 1) * 128, R], F8, kind="ExternalInput")
    adjF_d = nc.dram_tensor("adjF", [max(nb + nd, 1) * 128, R], F16, kind="ExternalInput")
    adjE_d = nc.dram_tensor("adjE", [max(ne, 1) * 128, R], F8, kind="ExternalInput")
    rho_d = nc.dram_tensor("rho", [1, R], F16, kind="ExternalInput")
    hext_d = nc.dram_tensor("hext", [128, NG * DE], F16, kind="ExternalInput")
    p2_d = nc.dram_tensor("p2", [128, NG], F32, kind="ExternalInput")
    n2_d = nc.dram_tensor("n2", [128, NG], F32, kind="ExternalInput")
    n2s_d = nc.dram_tensor("n2s", [128, NG], F32, kind="ExternalInput")
    b1_d = nc.dram_tensor("b1", [128, NG], F32, kind="ExternalInput")
    y_d = nc.dram_tensor("y", [R, DOUT], F32, kind="ExternalOutput")

    # processing order: round-robin across paths; global j-group ranges:
    # A: [0, na), B: [na, na+nb), D: [na+nb, na+nb+nd), E: [na+nb+nd, NG)
    counts = {p: n for p, n in (("A", na), ("B", nb), ("D", nd), ("E", ne)) if n}
    order = _interleave(counts)
    base = {"A": 0, "B": na, "D": na + nb, "E": na + nb + nd}
    steps = []
    seen = {"A": 0, "B": 0, "D": 0, "E": 0}
    for p in order:
        k = seen[p]
        seen[p] += 1
        steps.append((p, k, base[p] + k))

    with TileContext(nc) as tc:
        with (
            tc.tile_pool(name="consts", bufs=1) as consts,
            tc.tile_pool(name="slabs", bufs=1) as slabs,
            tc.tile_pool(name="tp", bufs=6) as tp,
            tc.tile_pool(name="qp", bufs=10) as qp,
            tc.tile_pool(name="tailp", bufs=1) as tailp,
        ):
            from concourse.masks import make_identity

            identity = consts.tile([128, 128], F32)
            make_identity(nc, identity)
            zcol = consts.tile([128, 1], F32)
            nc.vector.memset(zcol, 0.0)

            # small consts first: they unblock every compute path
            p2t = consts.tile([128, NG], F32)
            nc.sync.dma_start(out=p2t, in_=p2_d[:, :])
            n2t = consts.tile([128, NG], F32)
            nc.sync.dma_start(out=n2t, in_=n2_d[:, :])
            n2st = consts.tile([128, NG], F32)
            nc.sync.dma_start(out=n2st, in_=n2s_d[:, :])
            b1t = consts.tile([128, NG], F32)
            nc.sync.dma_start(out=b1t, in_=b1_d[:, :])
            rho_bc = consts.tile([128, R], F16)
            rho_ap = rho_d[:, :]
            nc.sync.dma_start(
                out=rho_bc,
                in_=bass.AP(tensor=rho_ap.tensor, offset=0, ap=[[0, 128], [1, R]]),
            )

            # adjacency slabs, chunked DMAs in consumption order; hext goes
            # after the first adj chunks (PE lags producers anyway)
            adjA_t = slabs.tile([128, max(na, 1), R], F8)
            adjF_t = slabs.tile([128, max(nb + nd, 1), R], F16)
            adjE_t = slabs.tile([128, max(ne, 1), R], F8)
            slab_of = {"A": 0, "B": 1, "D": 1, "E": 2}
            local = {"A": 0, "B": 0, "D": nb, "E": 0}
            dma_seen = set()
            dma_order = []
            for p, k, g in steps:
                key = (slab_of[p], (local[p] + k) // CHUNK)
                if key not in dma_seen:
                    dma_seen.add(key)
                    dma_order.append(key)
            hext = consts.tile([128, NG, DE], F16)
            slab_d = [adjA_d, adjF_d, adjE_d]
            slab_t = [adjA_t, adjF_t, adjE_t]
            slab_n = [na, nb + nd, ne]
            for di, (s, c) in enumerate(dma_order):
                g0, g1 = c * CHUNK, min((c + 1) * CHUNK, slab_n[s])
                nc.sync.dma_start(
                    out=slab_t[s][:, g0:g1, :],
                    in_=slab_d[s][g0 * 128 : g1 * 128, :].rearrange(
                        "(g p) i -> p g i", p=128
                    ),
                )
                if di == 1:
                    nc.sync.dma_start(
                        out=hext.rearrange("p g d -> p (g d)"), in_=hext_d[:, :]
                    )

            with tc.tile_pool(name="psacc", bufs=1, space="PSUM") as psacc:
                acc = psacc.tile([DE, R], F32)
                qs = [None] * len(steps)       # q tile per step
                dq = []                        # pending D-path (si, k, m) masks
                nsteps = len(steps)
                mm_done = 0

                def emit_mm(si):
                    p, k, g = steps[si]
                    for n0 in range(0, R, 512):
                        nc.tensor.matmul(
                            acc[:, n0 : n0 + 512],
                            hext[:, g, :],
                            qs[si][:, n0 : n0 + 512],
                            start=(si == 0 and n0 == 0),
                            stop=(si == nsteps - 1 and n0 == R - 512),
                        )

                def emit_dmask():
                    si, k, m = dq.pop(0)
                    q = qp.tile([128, R], F16, name="q")
                    nc.vector.tensor_tensor(q, m, adjF_t[:, nb + k, :], OP.mult)
                    qs[si] = q

                for si, (p, k, g) in enumerate(steps):
                    p2c = p2t[:, g : g + 1]
                    n2c = n2t[:, g : g + 1]
                    if p == "A":
                        t = tp.tile([128, R], F16, name="tA")
                        nc.vector.tensor_scalar_mul(t, rho_bc, n2c)
                        q = qp.tile([128, R], F16, name="q")
                        nc.gpsimd.scalar_tensor_tensor(
                            q, t, p2c, adjA_t[:, k, :], op_sel, OP.mult
                        )
                        qs[si] = q
                    elif p == "B":
                        m = tp.tile([128, R], F16, name="mB")
                        nc.vector.tensor_scalar(
                            m, rho_bc, n2c, p2c, OP.mult, op_sel
                        )
                        q = qp.tile([128, R], F16, name="q")
                        nc.vector.tensor_tensor(q, m, adjF_t[:, k, :], OP.mult)
                        qs[si] = q
                    elif p == "E":
                        q = qp.tile([128, R], F16, name="q")
                        nc.vector._custom_dve(
                            mmx, out=q, in0=adjE_t[:, k, :], in1=rho_bc,
                            s0=p2c, s1=n2c, imm2=0.0,
                        )
                        qs[si] = q
                    else:  # D: ACT producers now, mask op lagged
                        v = tp.tile([128, R], F16, name="vD")
                        nc.scalar.activation(
                            v, rho_bc, AF.Relu,
                            bias=b1t[:, g : g + 1], scale=n2st[:, g : g + 1],
                        )
                        m = tp.tile([128, R], F16, name="mD")
                        nc.scalar.activation(
                            m, v, AF.Identity, bias=p2c, scale=mid_sign
                        )
                        dq.append((si, k, m))
                        if len(dq) > D_LAG:
                            emit_dmask()
                    # PE lags; emit any matmul whose q is ready in step order
                    while mm_done < si - PE_LAG + 1 and qs[mm_done] is not None:
                        emit_mm(mm_done)
                        mm_done += 1
                while dq:
                    emit_dmask()
                while mm_done < nsteps:
                    assert qs[mm_done] is not None
                    emit_mm(mm_done)
                    mm_done += 1

                # tail: normalize + elu, back to i-major. hpT row DOUT holds
                # 1/s; ops emitted engine-major so the chunks pipeline.
                hpT = consts.tile([DE, R], F32)
                with tc.tile_pool(name="pstail", bufs=1, space="PSUM") as pstail:
                    nc.scalar.copy(hpT[0:DOUT, :], acc[0:DOUT, :])
                    nc.vector.reciprocal(hpT[DOUT:DE, :], acc[DOUT:DE, :])
                    ncc = R // 128
                    tpw = pstail.tile([128, ncc, DE], F32)
                    for cc in range(ncc):
                        nc.tensor.transpose(
                            tpw[:, cc, :],
                            hpT[:, cc * 128 : (cc + 1) * 128],
                            identity[0:DE, 0:DE],
                        )
                    vms, e2s = [], []
                    for cc in range(ncc):
                        # elu(v) = max(v, exp(min(v,0)) - 1), v = hp * (1/s)
                        vm = tailp.tile([128, DOUT], F32, name=f"vm{cc}")
                        nc.vector.tensor_scalar(
                            vm, tpw[:, cc, 0:DOUT], tpw[:, cc, DOUT:DE], 0.0,
                            OP.mult, OP.min,
                        )
                        vms.append(vm)
                    for cc in range(ncc):
                        e2 = tailp.tile([128, DOUT], F32, name=f"e2{cc}")
                        nc.scalar.activation(e2, vms[cc], AF.Exp, bias=zcol[:, 0:1])
                        e2s.append(e2)
                    for cc in range(ncc):
                        ysb = tailp.tile([128, DOUT], F32, name=f"y{cc}")
                        nc.vector._custom_dve(
                            emx, out=ysb, in0=tpw[:, cc, 0:DOUT], in1=e2s[cc],
                            s0=tpw[:, cc, DOUT:DE], s1=0.0, imm2=0.0,
                        )
                        nc.sync.dma_start(
                            out=y_d[cc * 128 : (cc + 1) * 128, :], in_=ysb
                        )
    nc.compile()
    return nc


def _splits(L1):
    if L1 >= 0.0:
        return 26, 14, 17, 7
    return 26, 21, 17, 0  # no fused-max path in the min variant


def _run(x, adj, w, a, a_coeff, b_coeff, c_coeff, d_coeff):
    global LAST_RESULTS, LAST_NC
    n, din = x.shape
    dout = w.shape[1]
    assert (n, din, dout) == (N, DIN, DOUT) and adj.shape == (N, N)

    A = float(np.asarray(a_coeff).reshape(-1)[0])
    B = float(np.asarray(b_coeff).reshape(-1)[0])
    C = float(np.asarray(c_coeff).reshape(-1)[0])
    D0 = float(np.asarray(d_coeff).reshape(-1)[0])
    L1 = _leaky(A + B)

    x = np.ascontiguousarray(x, dtype=np.float64)
    adj = np.asarray(adj, dtype=np.float32)
    assert ((adj == 0.0) | (adj == 1.0)).all(), "adj must be binary"
    w = np.ascontiguousarray(w, dtype=np.float64)
    a = np.ascontiguousarray(a, dtype=np.float64)

    # host precompute: h and the factorized per-node vectors
    h = x @ w
    hz1 = h @ a[:dout, 0]
    hz2 = h @ a[dout:, 0]
    a_e = L1 * C
    b_e = SLOPE * L1 * C
    lp2 = a_e * hz2 + L1 * D0
    ln2 = b_e * hz2 + SLOPE * L1 * D0
    lrho = (b_e - a_e) * hz1
    assert abs(lrho).max() < 10.5, "rho exceeds f16 range"
    maxlog = max(lp2.max(), lrho.max() + ln2.max())
    lng = np.log(8192.0) - maxlog  # global scale: products <= 8192 in f16
    P2 = np.exp(lp2 + lng)
    N2 = np.exp(ln2 + lng)
    rho = np.exp(lrho)

    if L1 >= 0.0:
        op_sel, mid_sign = OP.max, 1.0
        n2s, b1 = N2, -P2
    else:
        op_sel, mid_sign = OP.min, -1.0
        n2s, b1 = -N2, P2

    na, nb, nd, ne = _splits(L1)
    nc = _build(op_sel, mid_sign, na, nb, nd, ne)
    LAST_NC = nc

    # shared (j-indexed) tensors, [p, g] layout
    p2t = np.ascontiguousarray(P2.reshape(NG, 128).T, dtype=np.float32)
    n2t = np.ascontiguousarray(N2.reshape(NG, 128).T, dtype=np.float32)
    n2st = np.ascontiguousarray(n2s.reshape(NG, 128).T, dtype=np.float32)
    b1t = np.ascontiguousarray(b1.reshape(NG, 128).T, dtype=np.float32)
    # h_ext: [128, NG, DE] = h rows grouped by j-block, ones column appended
    hx = np.empty((128, NG, DE), dtype=np.float16)
    hx[:, :, 0:DOUT] = h.reshape(NG, 128, DOUT).transpose(1, 0, 2)
    hx[:, :, DOUT] = 1.0
    hx = np.ascontiguousarray(hx.reshape(128, NG * DE))

    nfr = na * 128                 # adjF row offset
    ner = (na + nb + nd) * 128     # adjE row offset
    in_maps = []
    for c in range(NCORES):
        sl = slice(c * R, (c + 1) * R)
        adjT = np.ascontiguousarray(adj[sl, :].T)  # [N j, R i]
        encE = (adjT[ner:] - 1.0) * BIG if ne else np.zeros((128, R), np.float32)
        in_maps.append(
            {
                "adjA": adjT[:nfr].astype(ml_dtypes.float8_e5m2)
                if na else np.zeros((128, R), ml_dtypes.float8_e5m2),
                "adjF": adjT[nfr:ner].astype(np.float16)
                if nb + nd else np.zeros((128, R), np.float16),
                "adjE": encE.astype(ml_dtypes.float8_e5m2),
                "rho": np.ascontiguousarray(
                    rho[sl].reshape(1, R).astype(np.float16)
                ),
                "hext": hx,
                "p2": p2t,
                "n2": n2t,
                "n2s": n2st,
                "b1": b1t,
            }
        )

    res = run_bass_kernel_spmd(
        nc, in_maps, core_ids=list(range(NCORES)), trace=TRACE
    )
    LAST_RESULTS = res
    return np.concatenate([r["y"] for r in res.results], axis=0).astype(np.float32)


def kernel(x, adj, w, a, a_coeff, b_coeff, c_coeff, d_coeff):
    return _run(x, adj, w, a, a_coeff, b_coeff, c_coeff, d_coeff)


# revision 7
# speedup vs baseline: 1.5755x; 1.1361x over previous
"""DGAT attention head on 8 trn2 NeuronCores.

Sharding: row-wise over query nodes (core c owns rows [c*R, (c+1)*R)).

Math (exact reparameterization of the reference):
  logit[i,j] = L1*leaky(z), z = C*(hz1_i+hz2_j)+D, L1 = leaky(A+B)
  exp(logit) factorizes per leaky branch; with L1>0 the branch select is a
  max, and softmax rows are scale-invariant, so dividing row i by
  exp(L1*C*hz1_i) leaves
      q[j,i] = adj[j,i] * max(P2[j], rho[i]*N2[j])
  with per-node vectors P2 = exp(a_e*hz2 + L1*D + lng),
  N2 = exp(b_e*hz2 + s*L1*D + lng), rho = exp((b_e-a_e)*hz1),
  a_e = L1*C, b_e = s*L1*C, s = 0.2 (lng: global scale for f16 range).
  No full-matrix exp/leaky remains; host precomputes h, rho, P2, N2.

Device pipeline per 128j x R megatile, split across four engine paths to
balance DVE / ACT / Pool / DMA occupancy (PE runs the shared accumulation):
  A: DVE  t = rho_bc * N2col            (f16 tensor_scalar, 4x mode)
     Pool q = max(t, P2col) * adjA      (scalar_tensor_tensor, fp8 adj)
  B: DVE  m = (rho_bc * N2col) max P2col  (dual-scalar tensor_scalar, 4x)
     DVE  q = m * adjF                  (tensor_tensor f16, 2x mode)
  D: ACT  v = Relu(n2s*rho_bc + b1)     (per-partition scale/bias cols)
     ACT  m = Identity(ms*v + P2col)
     DVE  q = m * adjF                  (tensor_tensor f16, 2x mode)
  E: DVE  q = max(max(P2col, rho_bc*N2col) + encE, 0)  (fused custom op,
     additive fp8 mask encoding {0, -BIG})
  PE: acc[65, R] += [h|1]^T_g @ q       (f16, start/stop over all 64 groups)
Emission is software-pipelined: D's mask op and the PE matmuls lag their
producers by a few steps so no in-order engine queue head-blocks on a
cross-engine dependency.
Tail: reciprocal of the sum row, PE transpose back to i-major,
  elu(v) = max(v, exp(min(v,0)) - 1), emitted engine-major.
"""

import numpy as np
import ml_dtypes

import concourse.bass as bass
import concourse.bacc as bacc
import concourse.mybir as mybir
import concourse.dve_ops as dve_ops
from concourse.dve_spec import Spec, Src0, Src1, C0, C1, C2, Zero, One, select, maxx
from concourse.tile import TileContext
from concourse.bass_utils import run_bass_kernel_spmd

F32 = mybir.dt.float32
F16 = mybir.dt.float16
F8 = mybir.dt.float8e5
AF = mybir.ActivationFunctionType
OP = mybir.AluOpType

NCORES = 8
SLOPE = 0.2  # leakyrelu negative slope (fixed in the reference)
BIG = 32768.0  # additive mask magnitude for path E (exact in fp8e5/f16)

# problem dims (hardcoded per spec)
N, DIN, DOUT = 8192, 256, 64
R = N // NCORES          # 1024 query rows per core
NG = N // 128            # 64 j-groups
DE = DOUT + 1            # h columns + ones column

CHUNK = 4                # j-groups per adjacency DMA
D_LAG = 1                # D-path mask op lags its ACT producers (in D-steps)
PE_LAG = 8               # matmuls lag q production (in steps)

TRACE = False
LAST_RESULTS = None
LAST_NC = None


def _leaky(z):
    return z if z >= 0.0 else SLOPE * z


def _finish_register(name, spec):
    from concourse.dve_spec import lower
    from concourse.dve_ops import has_src1
    from concourse.dve_uop import DveOpSpec

    op = dve_ops.DveOp(name, spec, subdim=False, uops_sha={})
    dve_ops.OPS.append(op)
    dve_ops.CUSTOM_DVE_SPECS[name] = spec
    dve_ops._SUB_OPCODE_FOR_NAME[name] = (
        dve_ops._CUSTOM_DVE_ROW_BASE + len(dve_ops.OPS) - 1
    )
    assert dve_ops._SUB_OPCODE_FOR_NAME[name] < 0x20
    for ver in ("v3",):
        pinned = DveOpSpec(
            name=name,
            opcode=dve_ops.get_dve_sub_opcode(name),
            uops=lower(spec, ver=ver),
            rd1_en=has_src1(spec),
        ).sha(ver)
        op.uops_sha[ver] = pinned
        dve_ops._COMPILE_CACHE.pop((name, ver), None)
        op.compile(ver)
    return op


def _register_elu_max_op():
    name = "ELU_MAX_ANT"
    for op in dve_ops.OPS:
        if op.name == name:
            return op
    spec = Spec(
        body=maxx(Src0 * C0, Src1 - One),
        reference=lambda in0, in1, s0, s1, imm2: np.maximum(
            in0 * s0, in1 - 1.0
        ).astype(np.float32),
    )
    return _finish_register(name, spec)


def _register_maxmax_op():
    name = "MAXMAX2_ANT"
    for op in dve_ops.OPS:
        if op.name == name:
            return op
    spec = Spec(
        body=maxx(maxx(C0, Src1 * C1) + Src0, Zero),
        reference=lambda in0, in1, s0, s1, imm2: np.maximum(
            np.maximum(s0, in1 * s1) + in0, 0.0
        ).astype(np.float32),
    )
    return _finish_register(name, spec)


def _interleave(counts):
    """Largest-remainder round-robin over path labels."""
    tot = sum(counts.values())
    acc = {k: 0.0 for k in counts}
    out = []
    for _ in range(tot):
        for k in counts:
            acc[k] += counts[k] / tot
        k = max(acc, key=lambda kk: acc[kk])
        acc[k] -= 1.0
        out.append(k)
    return out


def _build(op_sel, mid_sign, na, nb, nd, ne):
    """Build the SPMD Bass program (identical on all cores).

    op_sel: OP.max when L1 >= 0 else OP.min (branch-select direction).
    mid_sign: +1.0 / -1.0 scale on the D-path Identity op (min variant
    computes m = P2 - relu(P2 - rho*N2)). na/nb/nd/ne: path tile counts.
    """
    assert na + nb + nd + ne == NG
    emx = _register_elu_max_op()
    mmx = _register_maxmax_op()

    nc = bacc.Bacc("TRN2", target_bir_lowering=False)
    adjA_d = nc.dram_tensor("adjA", [max(na, 1) * 128, R], F8, kind="ExternalInput")
    adjF_d = nc.dram_tensor("adjF", [max(nb + nd, 1) * 128, R], F16, kind="ExternalInput")
    adjE_d = nc.dram_tensor("adjE", [max(ne, 1) * 128, R], F8, kind="ExternalInput")
    rho_d = nc.dram_tensor("rho", [1, R], F16, kind="ExternalInput")
    hext_d = nc.dram_tensor("hext", [128, NG * DE], F16, kind="ExternalInput")
    p2_d = nc.dram_tensor("p2", [128, NG], F32, kind="ExternalInput")
    n2_d = nc.dram_tensor("n2", [128, NG], F32, kind="ExternalInput")
    n2s_d = nc.dram_tensor("n2s", [128, NG], F32, kind="ExternalInput")
    b1_d = nc.dram_tensor("b1", [128, NG], F32, kind="ExternalInput")
    y_d = nc.dram_tensor("y", [R, DOUT], F32, kind="ExternalOutput")

    # processing order: round-robin across paths; global j-group ranges:
    # A: [0, na), B: [na, na+nb), D: [na+nb, na+nb+nd), E: [na+nb+nd, NG)
    counts = {p: n for p, n in (("A", na), ("B", nb), ("D", nd), ("E", ne)) if n}
    order = _interleave(counts)
    base = {"A": 0, "B": na, "D": na + nb, "E": na + nb + nd}
    steps = []
    seen = {"A": 0, "B": 0, "D": 0, "E": 0}
    for p in order:
        k = seen[p]
        seen[p] += 1
        steps.append((p, k, base[p] + k))

    with TileContext(nc) as tc:
        with (
            tc.tile_pool(name="consts", bufs=1) as consts,
            tc.tile_pool(name="slabs", bufs=1) as slabs,
            tc.tile_pool(name="tp", bufs=4) as tp,
            tc.tile_pool(name="qp", bufs=16) as qp,
            tc.tile_pool(name="tailp", bufs=1) as tailp,
        ):
            from concourse.masks import make_identity

            identity = consts.tile([128, 128], F32)
            make_identity(nc, identity)
            zcol = consts.tile([128, 1], F32)
            nc.vector.memset(zcol, 0.0)

            # small consts first: they unblock every compute path
            p2t = consts.tile([128, NG], F32)
            nc.sync.dma_start(out=p2t, in_=p2_d[:, :])
            n2t = consts.tile([128, NG], F32)
            nc.sync.dma_start(out=n2t, in_=n2_d[:, :])
            n2st = consts.tile([128, NG], F32)
            nc.sync.dma_start(out=n2st, in_=n2s_d[:, :])
            b1t = consts.tile([128, NG], F32)
            nc.sync.dma_start(out=b1t, in_=b1_d[:, :])
            rho_bc = consts.tile([128, R], F16)
            rho_ap = rho_d[:, :]
            nc.sync.dma_start(
                out=rho_bc,
                in_=bass.AP(tensor=rho_ap.tensor, offset=0, ap=[[0, 128], [1, R]]),
            )

            # adjacency slabs, chunked DMAs in consumption order; hext goes
            # after the first adj chunks (PE lags producers anyway)
            adjA_t = slabs.tile([128, max(na, 1), R], F8)
            adjF_t = slabs.tile([128, max(nb + nd, 1), R], F16)
            adjE_t = slabs.tile([128, max(ne, 1), R], F8)
            slab_of = {"A": 0, "B": 1, "D": 1, "E": 2}
            local = {"A": 0, "B": 0, "D": nb, "E": 0}
            dma_seen = set()
            dma_order = []
            for p, k, g in steps:
                key = (slab_of[p], (local[p] + k) // CHUNK)
                if key not in dma_seen:
                    dma_seen.add(key)
                    dma_order.append(key)
            hext = consts.tile([128, NG, DE], F16)
            slab_d = [adjA_d, adjF_d, adjE_d]
            slab_t = [adjA_t, adjF_t, adjE_t]
            slab_n = [na, nb + nd, ne]
            for di, (s, c) in enumerate(dma_order):
                g0, g1 = c * CHUNK, min((c + 1) * CHUNK, slab_n[s])
                nc.sync.dma_start(
                    out=slab_t[s][:, g0:g1, :],
                    in_=slab_d[s][g0 * 128 : g1 * 128, :].rearrange(
                        "(g p) i -> p g i", p=128
                    ),
                )
                if di == 1:
                    nc.sync.dma_start(
                        out=hext.rearrange("p g d -> p (g d)"), in_=hext_d[:, :]
                    )

            with tc.tile_pool(name="psacc", bufs=1, space="PSUM") as psacc:
                acc = psacc.tile([DE, R], F32)
                qs = [None] * len(steps)       # q tile per step
                dq = []                        # pending D-path (si, k, m) masks
                nsteps = len(steps)
                mm_done = 0

                def emit_mm(si):
                    p, k, g = steps[si]
                    for n0 in range(0, R, 512):
                        nc.tensor.matmul(
                            acc[:, n0 : n0 + 512],
                            hext[:, g, :],
                            qs[si][:, n0 : n0 + 512],
                            start=(si == 0 and n0 == 0),
                            stop=(si == nsteps - 1 and n0 == R - 512),
                        )

                def emit_dmask():
                    si, k, m = dq.pop(0)
                    q = qp.tile([128, R], F16, name="q")
                    nc.vector.tensor_tensor(q, m, adjF_t[:, nb + k, :], OP.mult)
                    qs[si] = q

                for si, (p, k, g) in enumerate(steps):
                    p2c = p2t[:, g : g + 1]
                    n2c = n2t[:, g : g + 1]
                    if p == "A":
                        t = tp.tile([128, R], F16, name="tA")
                        nc.vector.tensor_scalar_mul(t, rho_bc, n2c)
                        q = qp.tile([128, R], F16, name="q")
                        nc.gpsimd.scalar_tensor_tensor(
                            q, t, p2c, adjA_t[:, k, :], op_sel, OP.mult
                        )
                        qs[si] = q
                    elif p == "B":
                        m = tp.tile([128, R], F16, name="mB")
                        nc.vector.tensor_scalar(
                            m, rho_bc, n2c, p2c, OP.mult, op_sel
                        )
                        q = qp.tile([128, R], F16, name="q")
                        nc.vector.tensor_tensor(q, m, adjF_t[:, k, :], OP.mult)
                        qs[si] = q
                    elif p == "E":
                        q = qp.tile([128, R], F16, name="q")
                        nc.vector._custom_dve(
                            mmx, out=q, in0=adjE_t[:, k, :], in1=rho_bc,
                            s0=p2c, s1=n2c, imm2=0.0,
                        )
                        qs[si] = q
                    else:  # D: ACT producers now, mask op lagged
                        v = tp.tile([128, R], F16, name="vD")
                        nc.scalar.activation(
                            v, rho_bc, AF.Relu,
                            bias=b1t[:, g : g + 1], scale=n2st[:, g : g + 1],
                        )
                        m = tp.tile([128, R], F16, name="mD")
                        nc.scalar.activation(
                            m, v, AF.Identity, bias=p2c, scale=mid_sign
                        )
                        dq.append((si, k, m))
                        if len(dq) > D_LAG:
                            emit_dmask()
                    # PE lags; emit any matmul whose q is ready in step order
                    while mm_done < si - PE_LAG + 1 and qs[mm_done] is not None:
                        emit_mm(mm_done)
                        mm_done += 1
                while dq:
                    emit_dmask()
                while mm_done < nsteps:
                    assert qs[mm_done] is not None
                    emit_mm(mm_done)
                    mm_done += 1

                # tail: normalize + elu, back to i-major. hpT row DOUT holds
                # 1/s; ops emitted engine-major so the chunks pipeline.
                hpT = consts.tile([DE, R], F32)
                with tc.tile_pool(name="pstail", bufs=1, space="PSUM") as pstail:
                    nc.scalar.copy(hpT[0:DOUT, :], acc[0:DOUT, :])
                    nc.vector.reciprocal(hpT[DOUT:DE, :], acc[DOUT:DE, :])
                    ncc = R // 128
                    tpw = pstail.tile([128, ncc, DE], F32)
                    for cc in range(ncc):
                        nc.tensor.transpose(
                            tpw[:, cc, :],
                            hpT[:, cc * 128 : (cc + 1) * 128],
                            identity[0:DE, 0:DE],
                        )
                    vms, e2s = [], []
                    for cc in range(ncc):
                        # elu(v) = max(v, exp(min(v,0)) - 1), v = hp * (1/s)
                        vm = tailp.tile([128, DOUT], F32, name=f"vm{cc}")
                        nc.vector.tensor_scalar(
                            vm, tpw[:, cc, 0:DOUT], tpw[:, cc, DOUT:DE], 0.0,
                            OP.mult, OP.min,
                        )
                        vms.append(vm)
                    for cc in range(ncc):
                        e2 = tailp.tile([128, DOUT], F32, name=f"e2{cc}")
                        nc.scalar.activation(e2, vms[cc], AF.Exp, bias=zcol[:, 0:1])
                        e2s.append(e2)
                    for cc in range(ncc):
                        ysb = tailp.tile([128, DOUT], F32, name=f"y{cc}")
                        nc.vector._custom_dve(
                            emx, out=ysb, in0=tpw[:, cc, 0:DOUT], in1=e2s[cc],
                            s0=tpw[:, cc, DOUT:DE], s1=0.0, imm2=0.0,
                        )
                        nc.sync.dma_start(
                            out=y_d[cc * 128 : (cc + 1) * 128, :], in_=ysb
                        )
    nc.compile()
    return nc


def _splits(L1):
    if L1 >= 0.0:
        return 26, 14, 17, 7
    return 26, 21, 17, 0  # no fused-max path in the min variant


def _run(x, adj, w, a, a_coeff, b_coeff, c_coeff, d_coeff):
    global LAST_RESULTS, LAST_NC
    n, din = x.shape
    dout = w.shape[1]
    assert (n, din, dout) == (N, DIN, DOUT) and adj.shape == (N, N)

    A = float(np.asarray(a_coeff).reshape(-1)[0])
    B = float(np.asarray(b_coeff).reshape(-1)[0])
    C = float(np.asarray(c_coeff).reshape(-1)[0])
    D0 = float(np.asarray(d_coeff).reshape(-1)[0])
    L1 = _leaky(A + B)

    x = np.ascontiguousarray(x, dtype=np.float64)
    adj = np.asarray(adj, dtype=np.float32)
    assert ((adj == 0.0) | (adj == 1.0)).all(), "adj must be binary"
    w = np.ascontiguousarray(w, dtype=np.float64)
    a = np.ascontiguousarray(a, dtype=np.float64)

    # host precompute: h and the factorized per-node vectors
    h = x @ w
    hz1 = h @ a[:dout, 0]
    hz2 = h @ a[dout:, 0]
    a_e = L1 * C
    b_e = SLOPE * L1 * C
    lp2 = a_e * hz2 + L1 * D0
    ln2 = b_e * hz2 + SLOPE * L1 * D0
    lrho = (b_e - a_e) * hz1
    assert abs(lrho).max() < 10.5, "rho exceeds f16 range"
    maxlog = max(lp2.max(), lrho.max() + ln2.max())
    lng = np.log(8192.0) - maxlog  # global scale: products <= 8192 in f16
    P2 = np.exp(lp2 + lng)
    N2 = np.exp(ln2 + lng)
    rho = np.exp(lrho)

    if L1 >= 0.0:
        op_sel, mid_sign = OP.max, 1.0
        n2s, b1 = N2, -P2
    else:
        op_sel, mid_sign = OP.min, -1.0
        n2s, b1 = -N2, P2

    na, nb, nd, ne = _splits(L1)
    nc = _build(op_sel, mid_sign, na, nb, nd, ne)
    LAST_NC = nc

    # shared (j-indexed) tensors, [p, g] layout
    p2t = np.ascontiguousarray(P2.reshape(NG, 128).T, dtype=np.float32)
    n2t = np.ascontiguousarray(N2.reshape(NG, 128).T, dtype=np.float32)
    n2st = np.ascontiguousarray(n2s.reshape(NG, 128).T, dtype=np.float32)
    b1t = np.ascontiguousarray(b1.reshape(NG, 128).T, dtype=np.float32)
    # h_ext: [128, NG, DE] = h rows grouped by j-block, ones column appended
    hx = np.empty((128, NG, DE), dtype=np.float16)
    hx[:, :, 0:DOUT] = h.reshape(NG, 128, DOUT).transpose(1, 0, 2)
    hx[:, :, DOUT] = 1.0
    hx = np.ascontiguousarray(hx.reshape(128, NG * DE))

    nfr = na * 128                 # adjF row offset
    ner = (na + nb + nd) * 128     # adjE row offset
    in_maps = []
    for c in range(NCORES):
        sl = slice(c * R, (c + 1) * R)
        adjT = np.ascontiguousarray(adj[sl, :].T)  # [N j, R i]
        encE = (adjT[ner:] - 1.0) * BIG if ne else np.zeros((128, R), np.float32)
        in_maps.append(
            {
                "adjA": adjT[:nfr].astype(ml_dtypes.float8_e5m2)
                if na else np.zeros((128, R), ml_dtypes.float8_e5m2),
                "adjF": adjT[nfr:ner].astype(np.float16)
                if nb + nd else np.zeros((128, R), np.float16),
                "adjE": encE.astype(ml_dtypes.float8_e5m2),
                "rho": np.ascontiguousarray(
                    rho[sl].reshape(1, R).astype(np.float16)
                ),
                "hext": hx,
                "p2": p2t,
                "n2": n2t,
                "n2s": n2st,
                "b1": b1t,
            }
        )

    res = run_bass_kernel_spmd(
        nc, in_maps, core_ids=list(range(NCORES)), trace=TRACE
    )
    LAST_RESULTS = res
    return np.concatenate([r["y"] for r in res.results], axis=0).astype(np.float32)


def kernel(x, adj, w, a, a_coeff, b_coeff, c_coeff, d_coeff):
    return _run(x, adj, w, a, a_coeff, b_coeff, c_coeff, d_coeff)
